# revision 30
# baseline (speedup 1.0000x reference)
"""Trainium2 Bass kernel for nn_CNN_80221399155117.

Pipeline: full-vocab softmax -> token-prob gather -> -log2 surprisal ->
concat(hidden, surp) -> Conv1d(k=5, pad=2) -> MaxPool1d(5) -> ReLU -> FC.

Sharding: 8 cores = (batch b, seq-half h). Each core owns the pool-aligned
conv-output range [510h, 510h+510) of its batch, needing feats rows
[510h-2, 510h+512) (EXT=514, zero-padded outside [0,1024)). The softmax
normalizer is computed locally per row (positions sharded, vocab local),
so no collectives are needed.

Host prep does layout/dtype work only (shard, pad, transpose, cast to
fp16); all math runs on device. The sum-exp work is split between the
scalar (ACT) engine (exact exp with free accumulation) and the vector
(DVE) engine, which computes exp via the fp16 exponent-bits trick:
i16 = round(1024*(x*log2e + 15) + C); bitcast i16->f16 ~= e^x with a
mean-zero sawtooth error ~1.7% rms that averages out in the 32k-wide
sum (<0.1% on the normalizer, ~1e-3 bits on surprisal).
"""

import numpy as np

B, S, V, H = 4, 1024, 32000, 2048
OC, K = 128, 5
N_CORES = 8
Y_LOC = 510            # conv output positions per core (102 pool windows)
PO_LOC = 102           # pooled cols per core
EXT = 514              # feats rows incl conv halo (510 + 2 + 2)
NT = 4                 # main row tiles of 128
NHALO = EXT - 512      # 2 halo rows, vocab packed across partitions
HQ = 128 // NHALO      # partitions per halo row
HF = V // HQ           # free elems per partition for halo exp
LOG2E = 1.4426950408889634

CA, NA = 4800, 4       # ACT engine: 4 vocab chunks of 4800 per row tile
CD, ND = 6400, 2       # DVE engine: 2 vocab chunks of 6400 per row tile
VA = CA * NA           # 19200 (+ 12800 = 32000)
SCH_C = -59.0          # Schraudolph constant (mean-zero sum error)
LOG2_C = -127.0 + 0.0573   # exponent-bits log2 centering

_CACHE = {}


def _build_program():
    import concourse.tile as tile
    from concourse import bacc, bass, mybir

    f32 = mybir.dt.float32
    f16 = mybir.dt.float16
    i16 = mybir.dt.int16
    i32 = mybir.dt.int32
    Alu = mybir.AluOpType
    Act = mybir.ActivationFunctionType

    nc = bacc.Bacc("TRN2", target_bir_lowering=False, debug=False,
                   num_devices=N_CORES)

    lg = nc.dram_tensor("lg16", [EXT, V], f16, kind="ExternalInput").ap()
    xtd = nc.dram_tensor("xt16", [128, 16 * EXT], f16, kind="ExternalInput").ap()
    wtd = nc.dram_tensor("wt16", [128, 16 * K * OC], f16, kind="ExternalInput").ap()
    idxd = nc.dram_tensor("idx_loc", [128 * (NT + 1)], i32, kind="ExternalInput").ap()
    maskd = nc.dram_tensor("mask_loc", [128 * (NT + 1)], f32, kind="ExternalInput").ap()
    wsurp = nc.dram_tensor("wsurp", [1, K * OC], f16, kind="ExternalInput").ap()
    convb = nc.dram_tensor("convb", [OC, 1], f32, kind="ExternalInput").ap()
    fcw = nc.dram_tensor("fcw", [OC, 3 * PO_LOC], f32, kind="ExternalInput").ap()
    sentv = nc.dram_tensor("sentv", [128, 1], f32, kind="ExternalInput").ap()
    sentw = nc.dram_tensor("sentw", [128, 3], f32, kind="ExternalInput").ap()
    fcb = nc.dram_tensor("fcb", [3, 1], f32, kind="ExternalInput").ap()
    out = nc.dram_tensor("out_loc", [3, 1], f32, kind="ExternalOutput").ap()

    lg_flat = bass.AP(lg.tensor, 0, [[1, EXT * V], [1, 1]])

    with tile.TileContext(nc) as tc:
        with (
            tc.tile_pool(name="lpa", bufs=4) as lpa,        # ACT chunks
            tc.tile_pool(name="lpd", bufs=3) as lpd,        # DVE chunks + halo
            tc.tile_pool(name="scr", bufs=2) as scr,        # ACT exp scratch
            tc.tile_pool(name="yi", bufs=2) as yip,         # DVE i16 scratch
            tc.tile_pool(name="big", bufs=1) as big,        # resident tiles
            tc.tile_pool(name="sm", bufs=12) as sm,         # small stats
            tc.tile_pool(name="ps_y", bufs=1, space="PSUM") as ps_y,
            tc.tile_pool(name="ps_o", bufs=2, space="PSUM") as ps_o,
        ):
            # ---- first logits chunks kick off the stream (sync queue) ----
            xa0 = lpa.tile([128, CA], f16, tag="xa")
            nc.sync.dma_start(out=xa0[:], in_=lg[0:128, 0:CA])
            xd0 = lpd.tile([128, CD], f16, tag="xd")
            nc.sync.dma_start(out=xd0[:], in_=lg[0:128, VA:VA + CD])

            # halo rows (vocab packed) + resident tiles on the scalar HWDGE q
            hx = lpd.tile([128, HF], f16, tag="hx")
            halo_src = bass.AP(lg.tensor, 512 * V,
                               [[V, NHALO], [HF, HQ], [1, HF]])
            nc.scalar.dma_start(out=hx[:], in_=halo_src)
            xt = big.tile([128, 16 * EXT], f16, tag="xt")
            nc.scalar.dma_start(out=xt[:], in_=xtd)
            wtile = big.tile([128, 16 * K * OC], f16, tag="wtile")
            nc.scalar.dma_start(out=wtile[:], in_=wtd)

            # ---- small consts + gathers on SWDGE ----
            idx_sb = sm.tile([128, NT + 1], i32, tag="idx")
            nc.gpsimd.dma_start(out=idx_sb[:],
                                in_=bass.AP(idxd.tensor, 0, [[1, 128], [128, NT + 1]]))
            m_all = sm.tile([128, NT + 1], f32, tag="m")
            nc.gpsimd.dma_start(out=m_all[:],
                                in_=bass.AP(maskd.tensor, 0, [[1, 128], [128, NT + 1]]))
            g16 = sm.tile([128, NT + 1], f16, tag="g16")
            for t in range(NT + 1):
                # HW DGE honors only one index per partition per transfer
                nc.gpsimd.indirect_dma_start(
                    out=g16[:, t:t + 1], out_offset=None, in_=lg_flat,
                    in_offset=bass.IndirectOffsetOnAxis(
                        ap=idx_sb[:, t:t + 1], axis=0))
            wsurp_sb = big.tile([1, K * OC], f16, tag="wsurp")
            nc.gpsimd.dma_start(out=wsurp_sb[:], in_=wsurp)
            convb_sb = big.tile([OC, 1], f32, tag="convb")
            nc.gpsimd.dma_start(out=convb_sb[:], in_=convb)
            fcw_sb = big.tile([OC, 3 * PO_LOC], f32, tag="fcw")
            nc.gpsimd.dma_start(out=fcw_sb[:], in_=fcw)
            sentv_sb = big.tile([128, 1], f32, tag="sentv")
            nc.gpsimd.dma_start(out=sentv_sb[:], in_=sentv)
            sentw_sb = big.tile([128, 3], f32, tag="sentw")
            nc.gpsimd.dma_start(out=sentw_sb[:], in_=sentw)
            fcb_sb = big.tile([3, 1], f32, tag="fcb")
            nc.gpsimd.dma_start(out=fcb_sb[:], in_=fcb)

            ones_sb = big.tile([128, 1], f32, tag="ones")
            nc.vector.memset(ones_sb[:], 1.0)
            se_all = big.tile([128, NT + 1], f32, tag="se")
            nc.vector.memset(se_all[:, NT:], 1.0)   # -> log2 0 on unused lanes
            hsel = big.tile([128, NHALO], f32, tag="hsel")
            nc.vector.memset(hsel[:], 0.0)
            for a in range(NHALO):
                nc.vector.memset(hsel[a * HQ:(a + 1) * HQ, a:a + 1], 1.0)

            # g2 = g*log2e (filled late, after the gathers, off the DVE path)
            g2 = sm.tile([128, NT + 1], f32, tag="g2")

            # ---- halo: exp + partition-sum via selector matmul ----
            hscr = scr.tile([128, HF], f16, tag="he")
            hsums = sm.tile([128, 1], f32, tag="hsums")
            nc.scalar.activation(out=hscr[:], in_=hx[:], func=Act.Exp,
                                 accum_out=hsums[:])
            psum_h = ps_o.tile([NHALO, 1], f32, tag="ph")
            nc.tensor.matmul(out=psum_h[:], lhsT=hsel[:], rhs=hsums[:],
                             start=True, stop=True)
            nc.vector.tensor_copy(out=se_all[:NHALO, NT:], in_=psum_h[:])

            # surp row, collapsed from per-tile columns as they finish
            surp16 = big.tile([128, NT + 1], f16, tag="surp16")
            srow = big.tile([1, EXT], f16, tag="srow")
            lse_c = sm.tile([128, NT + 1], f32, tag="lse")

            def surp_tail(t):
                """log2(se) via exponent bits, minus g*log2e, times mask;
                then collapse column into srow (SWDGE, off-stream)."""
                nc.vector.tensor_scalar(
                    out=lse_c[:, t:t + 1], in0=se_all[:, t:t + 1].bitcast(i32),
                    scalar1=float(2.0 ** -23), scalar2=float(LOG2_C),
                    op0=Alu.mult, op1=Alu.add)
                nc.vector.tensor_tensor(
                    out=lse_c[:, t:t + 1], in0=lse_c[:, t:t + 1],
                    in1=g2[:, t:t + 1], op=Alu.subtract)
                nc.vector.tensor_tensor(
                    out=surp16[:, t:t + 1], in0=lse_c[:, t:t + 1],
                    in1=m_all[:, t:t + 1], op=Alu.mult)
                if t < NT:
                    nc.gpsimd.dma_start(out=srow[0:1, 128 * t:128 * (t + 1)],
                                        in_=surp16[:, t:t + 1])
                else:
                    nc.gpsimd.dma_start(out=srow[0:1, 512:EXT],
                                        in_=surp16[:NHALO, NT:NT + 1])

            # ---- conv: 80 hidden matmuls accumulate into one PSUM bank ----
            psum_y = ps_y.tile([OC, Y_LOC], f32, tag="y")
            first = True
            for cc in range(16):
                for k in range(K):
                    nc.tensor.matmul(
                        out=psum_y[:],
                        lhsT=wtile[:, cc * 640 + k * 128: cc * 640 + (k + 1) * 128],
                        rhs=xt[:, cc * EXT + k: cc * EXT + k + Y_LOC],
                        start=first,
                        stop=False,
                    )
                    first = False

            # ---- main loop: ACT (exact) + DVE (bit-trick) share the vocab ----
            sums_t = []
            for t in range(NT):
                r0 = 128 * t
                # DMA issue order: a0 d0 a1 a2 d1 a3 (tile 0: a0/d0 pre-issued)
                xa = []
                xd = []
                if t == 0:
                    xa.append(xa0)
                    xd.append(xd0)
                else:
                    xa_0 = lpa.tile([128, CA], f16, tag="xa")
                    xa.append(xa_0)
                    nc.sync.dma_start(out=xa_0[:], in_=lg[r0:r0 + 128, 0:CA])
                    xd_0 = lpd.tile([128, CD], f16, tag="xd")
                    xd.append(xd_0)
                    nc.sync.dma_start(out=xd_0[:],
                                      in_=lg[r0:r0 + 128, VA:VA + CD])
                for ci in (1, 2):
                    xa_ci = lpa.tile([128, CA], f16, tag="xa")
                    xa.append(xa_ci)
                    nc.sync.dma_start(
                        out=xa_ci[:],
                        in_=lg[r0:r0 + 128, ci * CA:(ci + 1) * CA])
                xd_1 = lpd.tile([128, CD], f16, tag="xd")
                xd.append(xd_1)
                nc.sync.dma_start(out=xd_1[:],
                                  in_=lg[r0:r0 + 128, VA + CD:VA + 2 * CD])
                xa_3 = lpa.tile([128, CA], f16, tag="xa")
                xa.append(xa_3)
                nc.sync.dma_start(out=xa_3[:],
                                  in_=lg[r0:r0 + 128, 3 * CA:4 * CA])

                sums = sm.tile([128, NA + ND], f32, tag="sums")
                sums_t.append(sums)
                for ci in range(NA):
                    e_sb = scr.tile([128, CA], f16, tag="e")
                    nc.scalar.activation(
                        out=e_sb[:], in_=xa[ci][:], func=Act.Exp,
                        accum_out=sums[:, ci:ci + 1])
                for di in range(ND):
                    yi = yip.tile([128, CD], i16, tag="y")
                    nc.vector.tensor_scalar(
                        out=yi[:], in0=xd[di][:],
                        scalar1=float(1024.0 * LOG2E),
                        scalar2=float(15.0 * 1024.0 + SCH_C),
                        op0=Alu.mult, op1=Alu.add)
                    nc.vector.tensor_reduce(
                        out=sums[:, NA + di:NA + di + 1],
                        in_=yi[:].bitcast(f16),
                        axis=mybir.AxisListType.X, op=Alu.add)
                if t == 1:
                    # gathers have landed by now; halo surp off critical path
                    nc.vector.tensor_scalar(out=g2[:], in0=g16[:],
                                            scalar1=LOG2E, scalar2=None,
                                            op0=Alu.mult)
                    surp_tail(NT)
                if t >= 1:
                    # previous tile's surp, overlapped with this tile's work
                    nc.vector.tensor_reduce(
                        out=se_all[:, t - 1:t], in_=sums_t[t - 1][:],
                        axis=mybir.AxisListType.X, op=Alu.add)
                    surp_tail(t - 1)

            nc.vector.tensor_reduce(
                out=se_all[:, NT - 1:NT], in_=sums_t[NT - 1][:],
                axis=mybir.AxisListType.X, op=Alu.add)
            surp_tail(NT - 1)

            # ---- surp channel: 5 contract-1 matmuls close the accum ----
            for k in range(K):
                nc.tensor.matmul(
                    out=psum_y[:],
                    lhsT=wsurp_sb[0:1, k * OC:(k + 1) * OC],
                    rhs=srow[0:1, k:k + Y_LOC],
                    start=False,
                    stop=(k == K - 1),
                )

            # ---- maxpool(5) + bias + relu ----
            pooled = big.tile([OC, PO_LOC], f32, tag="pooled")
            stop_off = K * (PO_LOC - 1) + 1
            nc.vector.tensor_copy(out=pooled[:], in_=psum_y[:, 0:stop_off:K])
            for j in range(1, K):
                nc.vector.tensor_tensor(out=pooled[:], in0=pooled[:],
                                        in1=psum_y[:, j:j + stop_off:K], op=Alu.max)
            nc.vector.tensor_scalar(out=pooled[:], in0=pooled[:],
                                    scalar1=convb_sb[:, 0:1], scalar2=None,
                                    op0=Alu.add)
            nc.vector.tensor_scalar(out=pooled[:], in0=pooled[:],
                                    scalar1=0.0, scalar2=None, op0=Alu.max)

            # ---- FC partial: red[oc, l] = sum_p pooled*fcw ----
            red = big.tile([OC, 3], f32, tag="red")
            fc_scr = big.tile([OC, PO_LOC], f32, tag="fcscr")
            for l in range(3):
                nc.vector.tensor_tensor(
                    out=fc_scr[:],
                    in0=pooled[:],
                    in1=fcw_sb[:, l * PO_LOC:(l + 1) * PO_LOC],
                    op=Alu.mult,
                )
                nc.vector.tensor_reduce(
                    out=red[:, l:l + 1], in_=fc_scr[:],
                    axis=mybir.AxisListType.X, op=Alu.add,
                )
            # sentiment branch (zeroed on h==1 cores)
            rs = sm.tile([128, 1], f32, tag="rs")
            nc.vector.tensor_scalar(out=rs[:], in0=sentv_sb[:], scalar1=0.0,
                                    scalar2=None, op0=Alu.max)
            tmp3 = sm.tile([128, 3], f32, tag="tmp3")
            nc.vector.tensor_scalar(out=tmp3[:], in0=sentw_sb[:],
                                    scalar1=rs[:, 0:1], scalar2=None, op0=Alu.mult)
            nc.vector.tensor_tensor(out=red[:], in0=red[:], in1=tmp3[:], op=Alu.add)

            psum_out = ps_o.tile([3, 1], f32, tag="po")
            nc.tensor.matmul(out=psum_out[:], lhsT=red[:], rhs=ones_sb[:],
                             start=True, stop=True)
            out_sb = sm.tile([3, 1], f32, tag="outsb")
            nc.vector.tensor_tensor(out=out_sb[:], in0=psum_out[:], in1=fcb_sb[:],
                                    op=Alu.add)
            nc.sync.dma_start(out=out, in_=out_sb[:])

    nc.compile()
    return nc


def _prep_core_inputs(core, input_ids, attention_mask, sentiment, logits,
                      hidden, conv_w, conv_b, fc_w, fc_b):
    b, h = core // 2, core % 2
    g0 = Y_LOC * h
    ext0 = g0 - 2

    lg = np.zeros((EXT, V), np.float16)
    idl = np.zeros((EXT,), np.int64)
    mk = np.zeros((EXT,), np.float32)
    hd = np.zeros((EXT, H), np.float32)
    lo = max(0, -ext0)            # local index where valid rows start
    s0, s1 = ext0 + lo, ext0 + EXT
    lg[lo:] = logits[b, s0:s1].astype(np.float16)
    idl[lo:] = input_ids[b, s0:s1]
    mk[lo:] = attention_mask[b, s0:s1]
    hd[lo:] = hidden[b, s0:s1]

    # conv weights, [128, 16*640] tile layout: block cc holds W[cc*128: , k, oc]
    wt = conv_w[:, :H, :].transpose(1, 2, 0).reshape(16, 128, K * OC)
    wt16 = np.ascontiguousarray(
        wt.astype(np.float16).transpose(1, 0, 2).reshape(128, 16 * K * OC))
    # hidden transposed into xt layout: [128 chan, 16 blocks * EXT pos]
    xt = hd.astype(np.float16).T.reshape(16, 128, EXT)
    xt16 = np.ascontiguousarray(xt.transpose(1, 0, 2).reshape(128, 16 * EXT))

    ws = np.ascontiguousarray(
        conv_w[:, H, :].T.astype(np.float16).reshape(1, K * OC))  # [1, K*OC]
    cb = np.ascontiguousarray(conv_b[:, None])                    # [OC, 1]

    w3 = fc_w[:, :OC * 204].reshape(3, OC, 204)
    fcw = np.ascontiguousarray(
        w3[:, :, h * PO_LOC:(h + 1) * PO_LOC].transpose(1, 0, 2).reshape(OC, 3 * PO_LOC))

    # flat gather indices, packed [p + 128*t] -> (128t+p)*V + id
    idx = np.zeros((128 * (NT + 1),), np.int32)
    for t in range(NT):
        rows = np.arange(128) + 128 * t
        idx[128 * t: 128 * (t + 1)] = (rows * V + idl[rows]).astype(np.int32)
    idx[128 * NT: 128 * NT + NHALO] = (
        (np.arange(NHALO) + 512) * V + idl[512:512 + NHALO]).astype(np.int32)

    mall = np.zeros((128 * (NT + 1),), np.float32)
    mall[:512] = mk[:512]
    mall[512:512 + NHALO] = mk[512:512 + NHALO]

    sv = np.zeros((128, 1), np.float32)
    sw = np.zeros((128, 3), np.float32)
    fb = np.zeros((3, 1), np.float32)
    if h == 0:
        sv[:3, 0] = sentiment[b]
        sw[:3, :] = fc_w[:, OC * 204:].T                   # [3 j, 3 l]
        fb[:, 0] = fc_b

    return {
        "lg16": lg, "xt16": xt16, "wt16": wt16,
        "idx_loc": idx, "mask_loc": mall,
        "wsurp": ws, "convb": cb, "fcw": fcw,
        "sentv": sv, "sentw": sw, "fcb": fb,
    }


def _install_ntff_hook():
    import sys
    import types
    try:
        import antenv
        from trn_agent_boot.trn_boot import _ntff_profile_via_ctypes
    except ImportError:
        return
    if "antenv.axon_hooks" in sys.modules:
        return
    mod = types.ModuleType("antenv.axon_hooks")
    _h = [None]
    mod.set_axon_ntff_profile_hook = lambda hk: _h.__setitem__(0, hk)
    mod.get_axon_ntff_profile_hook = lambda: _h[0]
    sys.modules["antenv.axon_hooks"] = mod
    antenv.axon_hooks = mod
    try:
        mod.set_axon_ntff_profile_hook(
            _ntff_profile_via_ctypes('/opt/axon/libaxon_pjrt.so'))
    except Exception:
        pass


def kernel(input_ids, attention_mask, sentiment, logits, hidden,
           conv_w, conv_b, fc_w, fc_b, _trace=False):
    from concourse.bass_utils import run_bass_kernel_spmd

    input_ids = np.asarray(input_ids)
    attention_mask = np.asarray(attention_mask, np.float32)
    sentiment = np.asarray(sentiment, np.float32)
    logits = np.asarray(logits, np.float32)
    hidden = np.asarray(hidden, np.float32)
    conv_w = np.asarray(conv_w, np.float32)
    conv_b = np.asarray(conv_b, np.float32)
    fc_w = np.asarray(fc_w, np.float32)
    fc_b = np.asarray(fc_b, np.float32)

    if "nc" not in _CACHE:
        _CACHE["nc"] = _build_program()
    nc = _CACHE["nc"]

    in_maps = [
        _prep_core_inputs(c, input_ids, attention_mask, sentiment, logits,
                          hidden, conv_w, conv_b, fc_w, fc_b)
        for c in range(N_CORES)
    ]
    if _trace:
        _install_ntff_hook()
    res = run_bass_kernel_spmd(nc, in_maps, list(range(N_CORES)), trace=_trace)
    _CACHE["last_result"] = res

    out = np.zeros((B, 3), np.float32)
    for b in range(B):
        out[b] = (res.results[2 * b]["out_loc"][:, 0]
                  + res.results[2 * b + 1]["out_loc"][:, 0])
    return out
